# revision 5
# baseline (speedup 1.0000x reference)
"""Trainium2 Bass kernel for the CRF mean-field layer (nn_CrfLayer).

Algorithm (C=2 classes, H=W=128, N=16384 pixels, 10 mean-field iterations):
  - softmax over 2 classes == sigmoid; sum-to-one lets us filter only class 0:
    sp1 = spatial_norm-normalized complement, bl1 = bnorm - bl0.
  - bilateral kernel K[i,j] = exp(-0.5*d2) computed via one 8-row augmented
    dot product on the PE: [f(5); -0.5|f|^2; 1; +/-1] x [f(5); 1; -0.5|f|^2; ls0]
    so that exp(psum) = K[i,j]*s0[j] directly (log s0 folded into the matmul).
  - sharding: core c owns columns i in [c*2048, (c+1)*2048) of K.  Setup builds
    that block once in fp16 into DRAM (and bnorm).  Each iteration the head of
    the j-range streams fp16 K tiles through the PE (moving operand) while the
    tail is recomputed on the fly with ScalarE exp+accumulate.  One 32KB
    AllGather per iteration shares the per-core bl0 shards; the cheap per-pixel
    work (softmax, separable spatial filter, q update) is replicated.
"""

import sys

sys.path.insert(0, "/opt/trn_rl_repo")

import numpy as np

H = 128
W = 128
C = 2
N = H * W
M = 8
BLK = N // M  # 2048
TA, TB, TG = 160.0, 3.0, 3.0
ITERS = 10

# j-range split: [0, N_DMA) streamed from DRAM fp16; [N_DMA, N) recomputed.
N_ACT = 6144
N_DMA = N - N_ACT
ICH = 512  # i-chunk width for the streamed matvec (one PSUM bank)
NG = BLK // ICH  # 4 i-groups per core
SUP = 16  # j-tiles (of 128 rows) per streaming DMA
KCH = N_ACT // 1024  # ScalarE exp chunks (1024 wide) per i-tile

_CACHE = {}


def _gauss1d(n, theta):
    d = np.arange(n, dtype=np.float32)
    return np.exp(-0.5 * ((d[:, None] - d[None, :]) / theta) ** 2).astype(np.float32)


def _build():
    import concourse.bass as bass
    import concourse.bacc as bacc
    from concourse import mybir, tile

    f32 = mybir.dt.float32
    f16 = mybir.dt.float16
    AF = mybir.ActivationFunctionType
    ALU = mybir.AluOpType
    AX = mybir.AxisListType

    nc = bacc.Bacc("TRN2", target_bir_lowering=False, debug=False, num_devices=M)

    gs_d = nc.declare_dram_parameter("gs", [H, H], f32, isOutput=False)
    isn_d = nc.declare_dram_parameter("inv_sn", [H, W], f32, isOutput=False)
    ident_d = nc.declare_dram_parameter("ident", [128, 128], f32, isOutput=False)
    grid_d = nc.declare_dram_parameter("gridT", [2, N], f32, isOutput=False)
    rgbT_d = nc.declare_dram_parameter("rgbT", [3, N], f32, isOutput=False)
    gridB_d = nc.declare_dram_parameter("gridB", [2, BLK], f32, isOutput=False)
    rgbB_d = nc.declare_dram_parameter("rgbB", [3, BLK], f32, isOutput=False)
    uin_d = nc.declare_dram_parameter("uin", [2, H, W], f32, isOutput=False)
    sw_d = nc.declare_dram_parameter("sw", [2, 2], f32, isOutput=False)
    bw_d = nc.declare_dram_parameter("bw", [2, 2], f32, isOutput=False)
    cm_d = nc.declare_dram_parameter("cm", [2, 2], f32, isOutput=False)
    cvec_d = nc.declare_dram_parameter("cvec", [3, N], f32, isOutput=False)
    qout_d = nc.declare_dram_parameter("qout", [2, H, W], f32, isOutput=True)

    rg = [list(range(M))]

    with tile.TileContext(nc) as tc:
        with (
            tc.tile_pool(name="pers", bufs=1) as pers,
            tc.tile_pool(name="dramP", bufs=1, space="DRAM") as dramP,
            tc.tile_pool(name="dram_ag", bufs=2, space="DRAM") as dram_ag,
            tc.tile_pool(name="psmall", bufs=2, space="PSUM") as psmall,
        ):
            gs = pers.tile([H, H], f32)
            isn = pers.tile([H, W], f32)
            ident = pers.tile([128, 128], f32)
            Hs8 = pers.tile([8, N], f32)
            G8 = pers.tile([8, BLK], f32)
            negc = pers.tile([128, 6], f32)
            U0m = pers.tile([H, W], f32)
            U1m = pers.tile([H, W], f32)
            q0 = pers.tile([H, W], f32)
            q1 = pers.tile([H, W], f32)
            inv_bn = pers.tile([H, W], f32)
            ones16 = pers.tile([128, 1], f16)
            fmean = pers.tile([5, 1], f32)

            K_dram = dramP.tile([NG, N, ICH], f16)

            nc.sync.dma_start(gs[:], gs_d[:])
            nc.sync.dma_start(isn[:], isn_d[:])
            nc.sync.dma_start(ident[:], ident_d[:])
            nc.vector.memset(ones16[:], 1.0)

            # ---------------- setup ----------------
            with (
                tc.tile_pool(name="ssb", bufs=1) as ssb,
                tc.tile_pool(name="psb", bufs=2, space="PSUM") as psb,
                tc.tile_pool(name="psbn", bufs=2, space="PSUM") as psbn,
                tc.tile_pool(name="k2p", bufs=4) as k2p,
            ):
                # feature rows (mean-centered): Hs8 = [f;1;-0.5sq;nls], G8 = [f;-0.5sq;1;-1]
                nc.sync.dma_start(Hs8[0:2, :], grid_d[:])
                nc.sync.dma_start(Hs8[2:5, :], rgbT_d[:])
                nc.vector.tensor_scalar_mul(Hs8[0:5, :], Hs8[0:5, :], 1.0 / 3.0)
                nc.vector.reduce_sum(fmean[:], Hs8[0:5, :], axis=AX.X)
                nc.vector.tensor_scalar_mul(fmean[:], fmean[:], 1.0 / N)
                nc.vector.tensor_scalar_sub(Hs8[0:5, :], Hs8[0:5, :], fmean[:])
                nc.sync.dma_start(Hs8[5:6, :], cvec_d[0:1, :])
                nc.sync.dma_start(Hs8[7:8, :], cvec_d[1:2, :])

                nc.sync.dma_start(G8[0:2, :], gridB_d[:])
                nc.sync.dma_start(G8[2:5, :], rgbB_d[:])
                nc.vector.tensor_scalar_mul(G8[0:5, :], G8[0:5, :], 1.0 / 3.0)
                nc.vector.tensor_scalar_sub(G8[0:5, :], G8[0:5, :], fmean[:])
                nc.sync.dma_start(G8[6:7, :], cvec_d[0:1, 0:BLK])
                nc.sync.dma_start(G8[7:8, :], cvec_d[2:3, 0:BLK])

                ones5 = ssb.tile([5, 1], f32)
                nc.sync.dma_start(ones5[:], cvec_d[0:1, 0:5].rearrange("a b -> b a"))
                for src, dst_row, nch in ((Hs8, 6, N // 512), (G8, 5, BLK // 512)):
                    for ch in range(nch):
                        sl = slice(ch * 512, (ch + 1) * 512)
                        sqc = ssb.tile([5, 512], f32, tag="sqc", name="sqc", bufs=3)
                        nc.vector.tensor_mul(sqc[:], src[0:5, sl], src[0:5, sl])
                        pssq = psmall.tile([1, 512], f32, tag="pss", name="pssq")
                        nc.tensor.matmul(pssq[:], ones5[:], sqc[:])
                        msqc = ssb.tile([1, 512], f32, tag="msqc", name="msqc", bufs=3)
                        nc.scalar.mul(msqc[:], pssq[:], -0.5)
                        nc.sync.dma_start(src[dst_row:dst_row + 1, sl], msqc[:])

                # unaries and q init
                nc.sync.dma_start(U0m[:], uin_d[0])
                nc.sync.dma_start(U1m[:], uin_d[1])
                nc.vector.tensor_copy(q0[:], U0m[:])
                nc.vector.tensor_copy(q1[:], U1m[:])

                # coefficients: A = cm@(sw[:,0]-sw[:,1]), B = cm@(bw[:,0]-bw[:,1]),
                # Cc = cm@(sw[:,1]+bw[:,1]);  q_c = (U_c - Cc_c) - A_c*sp0 - B_c*bl0n
                swt = ssb.tile([2, 2], f32)
                bwt = ssb.tile([2, 2], f32)
                cmT = ssb.tile([2, 2], f32)
                nc.sync.dma_start(swt[:], sw_d[:])
                nc.sync.dma_start(bwt[:], bw_d[:])
                nc.sync.dma_start(cmT[:], cm_d[:, :].rearrange("a b -> b a"))
                m3 = ssb.tile([2, 3], f32)
                nc.vector.tensor_sub(m3[:, 0:1], swt[:, 0:1], swt[:, 1:2])
                nc.vector.tensor_sub(m3[:, 1:2], bwt[:, 0:1], bwt[:, 1:2])
                nc.vector.tensor_add(m3[:, 2:3], swt[:, 1:2], bwt[:, 1:2])
                ps_c = psmall.tile([2, 3], f32, tag="pss", name="ps_c")
                nc.tensor.matmul(ps_c[:], cmT[:], m3[:])
                c23 = ssb.tile([2, 3], f32)
                nc.vector.tensor_copy(c23[:], ps_c[:])
                cflat_d = dramP.tile([1, 6], f32)
                nc.sync.dma_start(cflat_d[:], c23[:])
                cflat = ssb.tile([1, 6], f32)
                nc.sync.dma_start(cflat[:], cflat_d[:])
                ones_r = ssb.tile([1, 128], f32)
                nc.sync.dma_start(ones_r[:], cvec_d[0:1, 0:128])
                ps_b = psmall.tile([128, 6], f32, tag="pss", name="ps_b")
                nc.tensor.matmul(ps_b[:], ones_r[:], cflat[:])
                nc.vector.tensor_scalar_mul(negc[:], ps_b[:], -1.0)
                # U0m/U1m -> U - Cc   (negc col 2/5 hold -Cc)
                nc.vector.tensor_scalar(U0m[:], U0m[:], negc[:, 2:3], None, op0=ALU.add)
                nc.vector.tensor_scalar(U1m[:], U1m[:], negc[:, 5:6], None, op0=ALU.add)

                # K block build (fp16 -> DRAM) + bnorm
                bnflat = ssb.tile([1, BLK], f32)
                for g in range(NG):
                    psg = psbn.tile([1, 512], f32, tag="psbn", name="psg")
                    for pair in range(N // 256):
                        jt = 2 * pair
                        ps = psb.tile([128, 1024], f32, tag="psb", name="ps")
                        nc.tensor.matmul(
                            ps[:, 0:512],
                            Hs8[:, jt * 128:(jt + 1) * 128],
                            G8[:, g * 512:(g + 1) * 512],
                        )
                        nc.tensor.matmul(
                            ps[:, 512:1024],
                            Hs8[:, (jt + 1) * 128:(jt + 2) * 128],
                            G8[:, g * 512:(g + 1) * 512],
                        )
                        kt2 = k2p.tile([128, 1024], f16, tag="kt2", name="kt2")
                        nc.scalar.activation(kt2[:], ps[:], AF.Exp)
                        nc.tensor.matmul(
                            psg[:], ones16[:], kt2[:, 0:512],
                            start=(pair == 0), stop=False,
                        )
                        nc.tensor.matmul(
                            psg[:], ones16[:], kt2[:, 512:1024],
                            start=False, stop=(pair == N // 256 - 1),
                        )
                        nc.sync.dma_start(
                            K_dram[g, jt * 128:(jt + 2) * 128, :].rearrange(
                                "(jt p) i -> p jt i", p=128
                            ),
                            kt2[:, :].rearrange("p (jt i) -> p jt i", i=512),
                        )
                    nc.vector.tensor_copy(bnflat[:, g * 512:(g + 1) * 512], psg[:])

                # AllGather bnorm (row0 zeros, row1 bnorm) -> bnorm plane -> 1/bnorm
                agin0 = dram_ag.tile([2, BLK], f32, tag="agin", name="agin0")
                nc.sync.dma_start(agin0[0:1, :], cvec_d[1:2, 0:BLK])
                nc.sync.dma_start(agin0[1:2, :], bnflat[:])
                agout0 = dram_ag.tile(
                    [2 * M, BLK], f32, tag="agout", name="agout0", addr_space="Shared"
                )
                nc.gpsimd.collective_compute(
                    "AllGather", ALU.bypass, replica_groups=rg,
                    ins=[agin0.opt()], outs=[agout0.opt()],
                )
                bnp = ssb.tile([H, W], f32)
                for r in range(M):
                    nc.sync.dma_start(
                        bnp[r * 16:(r + 1) * 16, :],
                        agout0[2 * r + 1:2 * r + 2, :].rearrange(
                            "o (a b) -> (o a) b", a=16
                        ),
                    )
                nc.vector.reciprocal(inv_bn[:], bnp[:])

            # ---------------- iterations ----------------
            with (
                tc.tile_pool(name="isb", bufs=2) as isb,
                tc.tile_pool(name="kstr", bufs=3) as kstr,
                tc.tile_pool(name="psact", bufs=2, space="PSUM") as psact,
                tc.tile_pool(name="psdma", bufs=2, space="PSUM") as psdma,
            ):
                for t in range(ITERS):
                    # softmax pieces: s0 = 1/(1+e^(q1-q0)), nls = log(1+e^(q1-q0))
                    D = isb.tile([H, W], f32, tag="D", name="D")
                    nc.vector.tensor_sub(D[:], q1[:], q0[:])
                    E = isb.tile([H, W], f32, tag="E", name="E")
                    nc.scalar.activation(E[:], D[:], AF.Exp)
                    Uu = isb.tile([H, W], f32, tag="Uu", name="Uu")
                    nc.vector.tensor_scalar_add(Uu[:], E[:], 1.0)
                    S0 = isb.tile([H, W], f32, tag="S0", name="S0")
                    nc.vector.reciprocal(S0[:], Uu[:])
                    NLS = isb.tile([H, W], f32, tag="NLS", name="NLS")
                    nc.scalar.activation(NLS[:], Uu[:], AF.Ln)
                    # flatten nls into Hs8 row 7 (DRAM bounce keeps orders aligned)
                    nls_d = dramP.tile([1, N], f32, tag="nls_d", name="nls_d", bufs=2)
                    nc.sync.dma_start(nls_d[:], NLS[:])
                    nc.sync.dma_start(Hs8[7:8, :], nls_d[:])

                    # s0 transposed (fp16) = streamed-matvec weights
                    ps_t = psmall.tile([128, 128], f32, tag="pss", name="ps_t")
                    nc.tensor.transpose(ps_t[:], S0[:], ident[:])
                    s0T = isb.tile([128, 128], f16, tag="s0T", name="s0T")
                    nc.vector.tensor_copy(s0T[:], ps_t[:])

                    # spatial: sp0 = (Gs @ S0 @ Gs) * inv_sn   (Gs symmetric)
                    ps1 = psmall.tile([128, 128], f32, tag="pss", name="ps1")
                    nc.tensor.matmul(ps1[:], gs[:], S0[:])
                    T1 = isb.tile([H, W], f32, tag="T1", name="T1")
                    nc.vector.tensor_copy(T1[:], ps1[:])
                    ps2 = psmall.tile([128, 128], f32, tag="pss", name="ps2")
                    nc.tensor.transpose(ps2[:], T1[:], ident[:])
                    T1t = isb.tile([H, W], f32, tag="T1t", name="T1t")
                    nc.vector.tensor_copy(T1t[:], ps2[:])
                    ps3 = psmall.tile([128, 128], f32, tag="pss", name="ps3")
                    nc.tensor.matmul(ps3[:], gs[:], T1t[:])
                    T2t = isb.tile([H, W], f32, tag="T2t", name="T2t")
                    nc.vector.tensor_copy(T2t[:], ps3[:])
                    ps4 = psmall.tile([128, 128], f32, tag="pss", name="ps4")
                    nc.tensor.transpose(ps4[:], T2t[:], ident[:])
                    SP0 = isb.tile([H, W], f32, tag="SP0", name="SP0")
                    nc.vector.tensor_mul(SP0[:], ps4[:], isn[:])

                    # bilateral ACT path: exp(-0.5 d2 + log s0_j) accumulated over j
                    acc = isb.tile([128, 16 * KCH], f32, tag="acc", name="acc")
                    for it in range(16):
                        for kk in range(KCH):
                            pa = psact.tile([128, 1024], f32, tag="pa", name="pa")
                            j0 = N_DMA + kk * 1024
                            nc.tensor.matmul(
                                pa[:, 0:512],
                                G8[:, it * 128:(it + 1) * 128],
                                Hs8[:, j0:j0 + 512],
                            )
                            nc.tensor.matmul(
                                pa[:, 512:1024],
                                G8[:, it * 128:(it + 1) * 128],
                                Hs8[:, j0 + 512:j0 + 1024],
                            )
                            nc.scalar.activation(
                                pa[:], pa[:], AF.Exp,
                                accum_out=acc[:, it * KCH + kk:it * KCH + kk + 1],
                            )
                    blact = isb.tile([128, 16], f32, tag="blact", name="blact")
                    nc.vector.reduce_sum(
                        blact[:],
                        acc[:, :].rearrange("p (a b) -> p a b", b=KCH),
                        axis=AX.X,
                    )

                    # bilateral DMA path: stream fp16 K tiles, matvec with s0T cols
                    bldma = isb.tile([1, BLK], f32, tag="bldma", name="bldma")
                    njt = N_DMA // 128
                    for g in range(NG):
                        pd = psdma.tile([1, 512], f32, tag="pd", name="pd")
                        for sup in range(njt // SUP):
                            kt = kstr.tile([128, SUP * 512], f16, tag="kt", name="kt")
                            nc.sync.dma_start(
                                kt[:, :].rearrange("p (jt i) -> p jt i", i=512),
                                K_dram[
                                    g, sup * SUP * 128:(sup + 1) * SUP * 128, :
                                ].rearrange("(jt p) i -> p jt i", p=128),
                            )
                            for jl in range(SUP):
                                jt = sup * SUP + jl
                                nc.tensor.matmul(
                                    pd[:],
                                    s0T[:, jt:jt + 1],
                                    kt[:, jl * 512:(jl + 1) * 512],
                                    start=(jt == 0), stop=(jt == njt - 1),
                                )
                        nc.vector.tensor_copy(bldma[:, g * 512:(g + 1) * 512], pd[:])

                    # gather: row0 = act part (transposed to flat order), row1 = dma part
                    ps_bt = psmall.tile([16, 128], f32, tag="pss", name="ps_bt")
                    nc.tensor.transpose(ps_bt[:], blact[:], ident[:])
                    tbact = isb.tile([16, 128], f32, tag="tbact", name="tbact")
                    nc.vector.tensor_copy(tbact[:], ps_bt[:])
                    agin = dram_ag.tile([2, BLK], f32, tag="agin", name="agin")
                    nc.sync.dma_start(
                        agin[0:1, :].rearrange("o (a b) -> (o a) b", a=16), tbact[:]
                    )
                    nc.sync.dma_start(agin[1:2, :], bldma[:])
                    agout = dram_ag.tile(
                        [2 * M, BLK], f32, tag="agout", name="agout",
                        addr_space="Shared",
                    )
                    nc.gpsimd.collective_compute(
                        "AllGather", ALU.bypass, replica_groups=rg,
                        ins=[agin.opt()], outs=[agout.opt()],
                    )
                    APL = isb.tile([H, W], f32, tag="APL", name="APL")
                    DPL = isb.tile([H, W], f32, tag="DPL", name="DPL")
                    for r in range(M):
                        nc.sync.dma_start(
                            APL[r * 16:(r + 1) * 16, :],
                            agout[2 * r:2 * r + 1, :].rearrange(
                                "o (a b) -> (o a) b", a=16
                            ),
                        )
                        nc.sync.dma_start(
                            DPL[r * 16:(r + 1) * 16, :],
                            agout[2 * r + 1:2 * r + 2, :].rearrange(
                                "o (a b) -> (o a) b", a=16
                            ),
                        )
                    BL = isb.tile([H, W], f32, tag="BL", name="BL")
                    nc.vector.tensor_add(BL[:], APL[:], DPL[:])
                    BLN = isb.tile([H, W], f32, tag="BLN", name="BLN")
                    nc.vector.tensor_mul(BLN[:], BL[:], inv_bn[:])

                    # q update: q_c = (U_c - Cc_c) - A_c*sp0 - B_c*bl0n
                    t0 = isb.tile([H, W], f32, tag="t0", name="t0")
                    nc.vector.scalar_tensor_tensor(
                        t0[:], SP0[:], negc[:, 0:1], U0m[:], op0=ALU.mult, op1=ALU.add
                    )
                    nc.vector.scalar_tensor_tensor(
                        q0[:], BLN[:], negc[:, 1:2], t0[:], op0=ALU.mult, op1=ALU.add
                    )
                    t1 = isb.tile([H, W], f32, tag="t1", name="t1")
                    nc.vector.scalar_tensor_tensor(
                        t1[:], SP0[:], negc[:, 3:4], U1m[:], op0=ALU.mult, op1=ALU.add
                    )
                    nc.vector.scalar_tensor_tensor(
                        q1[:], BLN[:], negc[:, 4:5], t1[:], op0=ALU.mult, op1=ALU.add
                    )

            nc.sync.dma_start(qout_d[0], q0[:])
            nc.sync.dma_start(qout_d[1], q1[:])

    nc.compile()
    return nc


def _get_nc():
    if "nc" not in _CACHE:
        _CACHE["nc"] = _build()
    return _CACHE["nc"]


def kernel(**inputs):
    from concourse.bass_utils import run_bass_kernel_spmd

    unaries = np.ascontiguousarray(np.asarray(inputs["unaries"], dtype=np.float32))
    rgb = np.ascontiguousarray(np.asarray(inputs["rgb"], dtype=np.float32))
    sw = np.ascontiguousarray(np.asarray(inputs["spatial_ker_weights"], dtype=np.float32))
    bw = np.ascontiguousarray(np.asarray(inputs["bilateral_ker_weights"], dtype=np.float32))
    cm = np.ascontiguousarray(np.asarray(inputs["compatibility_matrix"], dtype=np.float32))

    gsm = _gauss1d(H, TG)
    rsum = gsm.sum(axis=1).astype(np.float32)
    inv_sn = (1.0 / np.outer(rsum, rsum)).astype(np.float32)
    ident = np.eye(128, dtype=np.float32)
    ys, xs = np.meshgrid(
        np.arange(H, dtype=np.float32), np.arange(W, dtype=np.float32), indexing="ij"
    )
    gridT = np.ascontiguousarray(
        np.stack([(xs * (3.0 / TA)).reshape(N), (ys * (3.0 / TA)).reshape(N)])
    ).astype(np.float32)
    rgbT = np.ascontiguousarray(rgb[0].reshape(N, 3).T).astype(np.float32)
    uin = np.ascontiguousarray(unaries[0].transpose(2, 0, 1)).astype(np.float32)

    cvec = np.stack([
        np.ones(N, np.float32), np.zeros(N, np.float32), -np.ones(N, np.float32)
    ])
    common = {
        "gs": gsm, "inv_sn": inv_sn, "ident": ident, "gridT": gridT, "rgbT": rgbT,
        "uin": uin, "sw": sw, "bw": bw, "cm": cm, "cvec": cvec,
    }
    in_maps = []
    for c in range(M):
        blk = slice(c * BLK, (c + 1) * BLK)
        m = dict(common)
        m["gridB"] = np.ascontiguousarray(gridT[:, blk])
        m["rgbB"] = np.ascontiguousarray(rgbT[:, blk])
        in_maps.append(m)

    nc = _get_nc()
    import os

    trace = bool(int(os.environ.get("BASS_KERNEL_TRACE", "0")))
    res = run_bass_kernel_spmd(nc, in_maps, list(range(M)), trace=trace)
    _CACHE["exec_time_ns"] = res.exec_time_ns
    _CACHE["results"] = res
    q = np.asarray(res.results[0]["qout"])  # [2, H, W]
    return np.ascontiguousarray(q.transpose(1, 2, 0))[None]  # (1, H, W, 2)


# revision 7
# speedup vs baseline: 1.7463x; 1.7463x over previous
"""Trainium2 Bass kernel for the CRF mean-field layer (nn_CrfLayer).

Algorithm (C=2 classes, H=W=128, N=16384 pixels, 10 mean-field iterations):
  - softmax over 2 classes == sigmoid; sum-to-one lets us filter only class 0:
    sp1 complement via spatial norm, bl1 = bnorm - bl0.
  - bilateral kernel K[i,j] = exp(-0.5*d2) via one augmented dot product on
    the PE; operands are split-fp16 (hi+lo) so the moving operand streams at
    full rate: exponent = f_i.f_j - 0.5|f_i|^2 - 0.5|f_j|^2 + log s0_j, so
    exp(psum) = K[i,j]*s0[j] directly (log s0 folded into the matmul).
  - sharding: core c owns columns i in [c*2048, (c+1)*2048) of K.  Setup
    builds that block once in fp16 into DRAM (and bnorm).  Each iteration the
    head of the j-range streams fp16 K tiles through the PE (moving operand)
    while the tail is recomputed on the fly with ScalarE exp+accumulate.  One
    32KB AllGather per iteration shares the per-core bl0 shards; the cheap
    per-pixel work (softmax, separable spatial filter, q update) is
    replicated on all cores.

split-fp16 contraction rows (k = 21), exponent = sum_k G[k,i] * Hs[k,j]:
   k0-4 : G fh_i    | Hs fh_j        k15: G msqh_i | Hs 1
   k5-9 : G fh_i    | Hs fl_j        k16: G msql_i | Hs 1
   k10-14: G fl_i   | Hs fh_j        k17: G 1      | Hs msqh_j
                                     k18: G 1      | Hs msql_j
                                     k19: G -1     | Hs nlsh_j   (-log s0 hi)
                                     k20: G -1     | Hs nlsl_j   (-log s0 lo)
"""

import sys

sys.path.insert(0, "/opt/trn_rl_repo")

import numpy as np

H = 128
W = 128
C = 2
N = H * W
M = 8
BLK = N // M  # 2048
TA, TB, TG = 160.0, 3.0, 3.0
ITERS = 10

# j-range split: [0, N_DMA) streamed from DRAM fp16; [N_DMA, N) recomputed.
N_ACT = 6144
N_DMA = N - N_ACT
ICH = 512  # i-chunk width for the streamed matvec (one PSUM bank)
NG = BLK // ICH  # 4 i-groups per core
KCH = N_ACT // 1024  # ScalarE exp chunks (1024 wide) per i-tile
KR = 21  # contraction rows

_CACHE = {}


def _gauss1d(n, theta):
    d = np.arange(n, dtype=np.float32)
    return np.exp(-0.5 * ((d[:, None] - d[None, :]) / theta) ** 2).astype(np.float32)


def _build():
    import concourse.bass as bass
    import concourse.bacc as bacc
    from concourse import mybir, tile

    f32 = mybir.dt.float32
    f16 = mybir.dt.float16
    AF = mybir.ActivationFunctionType
    ALU = mybir.AluOpType
    AX = mybir.AxisListType

    nc = bacc.Bacc("TRN2", target_bir_lowering=False, debug=False, num_devices=M)

    gs_d = nc.declare_dram_parameter("gs", [H, H], f32, isOutput=False)
    isn_d = nc.declare_dram_parameter("inv_sn", [H, W], f32, isOutput=False)
    ident_d = nc.declare_dram_parameter("ident", [128, 128], f32, isOutput=False)
    grid_d = nc.declare_dram_parameter("gridT", [2, N], f32, isOutput=False)
    rgbT_d = nc.declare_dram_parameter("rgbT", [3, N], f32, isOutput=False)
    gridB_d = nc.declare_dram_parameter("gridB", [2, BLK], f32, isOutput=False)
    rgbB_d = nc.declare_dram_parameter("rgbB", [3, BLK], f32, isOutput=False)
    uin_d = nc.declare_dram_parameter("uin", [2, H, W], f32, isOutput=False)
    sw_d = nc.declare_dram_parameter("sw", [2, 2], f32, isOutput=False)
    bw_d = nc.declare_dram_parameter("bw", [2, 2], f32, isOutput=False)
    cm_d = nc.declare_dram_parameter("cm", [2, 2], f32, isOutput=False)
    cvec_d = nc.declare_dram_parameter("cvec", [3, N], f32, isOutput=False)
    cv16_d = nc.declare_dram_parameter("cvec16", [3, N], f16, isOutput=False)
    qout_d = nc.declare_dram_parameter("qout", [2, H, W], f32, isOutput=True)

    rg = [list(range(M))]

    with tile.TileContext(nc) as tc:
        with (
            tc.tile_pool(name="pers", bufs=1) as pers,
            tc.tile_pool(name="dramP", bufs=1, space="DRAM") as dramP,
            tc.tile_pool(name="dram_ag", bufs=2, space="DRAM") as dram_ag,
            tc.tile_pool(name="psmall", bufs=2, space="PSUM") as psmall,
        ):
            gs = pers.tile([H, H], f32)
            isn = pers.tile([H, W], f32)
            ident = pers.tile([128, 128], f32)
            Hs16 = pers.tile([KR, N], f16)
            G16 = pers.tile([KR, BLK], f16)
            negc = pers.tile([128, 6], f32)
            U0m = pers.tile([H, W], f32)
            U1m = pers.tile([H, W], f32)
            q0 = pers.tile([H, W], f32)
            q1 = pers.tile([H, W], f32)
            inv_bn = pers.tile([H, W], f32)
            ones16 = pers.tile([128, 1], f16)

            K_dram = dramP.tile([NG, N, ICH], f16)

            nc.sync.dma_start(gs[:], gs_d[:])
            nc.sync.dma_start(isn[:], isn_d[:])
            nc.sync.dma_start(ident[:], ident_d[:])
            nc.sync.dma_start(ones16[:], cv16_d[0:1, 0:128].rearrange("a b -> b a"))

            # ---------------- setup ----------------
            with (
                tc.tile_pool(name="ssb", bufs=1) as ssb,
                tc.tile_pool(name="ssc", bufs=2) as ssc,
                tc.tile_pool(name="psb", bufs=2, space="PSUM") as psb,
                tc.tile_pool(name="psbn", bufs=2, space="PSUM") as psbn,
                tc.tile_pool(name="k2p", bufs=4) as k2p,
            ):
                # f32 features (mean-centered): rows = [3x/160, 3y/160, r, g, b]/3
                F5 = ssb.tile([5, N], f32)
                FB5 = ssb.tile([5, BLK], f32)
                fmean = ssb.tile([5, 1], f32)
                nc.sync.dma_start(F5[0:2, :], grid_d[:])
                nc.sync.dma_start(F5[2:5, :], rgbT_d[:])
                nc.vector.tensor_scalar_mul(F5[:], F5[:], 1.0 / 3.0)
                nc.vector.reduce_sum(fmean[:], F5[:], axis=AX.X)
                nc.vector.tensor_scalar_mul(fmean[:], fmean[:], 1.0 / N)
                nc.vector.tensor_scalar_sub(F5[:], F5[:], fmean[:])
                nc.sync.dma_start(FB5[0:2, :], gridB_d[:])
                nc.sync.dma_start(FB5[2:5, :], rgbB_d[:])
                nc.vector.tensor_scalar_mul(FB5[:], FB5[:], 1.0 / 3.0)
                nc.vector.tensor_scalar_sub(FB5[:], FB5[:], fmean[:])

                # split features into fp16 hi/lo and scatter into Hs16/G16 rows
                def split_rows(src, nsrc, dst, hi_rows, lo_rows):
                    nch = nsrc // 1024
                    for ch in range(nch):
                        sl = slice(ch * 1024, (ch + 1) * 1024)
                        hi = ssc.tile([5, 1024], f16, tag="sp_hi", name="hi")
                        nc.vector.tensor_copy(hi[:], src[:, sl])
                        hb = ssc.tile([5, 1024], f32, tag="sp_hb", name="hb")
                        nc.vector.tensor_copy(hb[:], hi[:])
                        lo32 = ssc.tile([5, 1024], f32, tag="sp_lo32", name="lo32")
                        nc.vector.tensor_sub(lo32[:], src[:, sl], hb[:])
                        lo = ssc.tile([5, 1024], f16, tag="sp_lo", name="lo")
                        nc.vector.tensor_copy(lo[:], lo32[:])
                        for r0 in hi_rows:
                            nc.sync.dma_start(dst[r0:r0 + 5, sl], hi[:])
                        for r0 in lo_rows:
                            nc.sync.dma_start(dst[r0:r0 + 5, sl], lo[:])

                split_rows(F5, N, Hs16, hi_rows=(0, 10), lo_rows=(5,))
                split_rows(FB5, BLK, G16, hi_rows=(0, 5), lo_rows=(10,))

                # msq rows: -0.5*|f|^2 split hi/lo
                ones5 = ssb.tile([5, 1], f32)
                nc.sync.dma_start(ones5[:], cvec_d[0:1, 0:5].rearrange("a b -> b a"))

                def msq_rows(src, nsrc, dst, hi_row, lo_row):
                    for ch in range(nsrc // 512):
                        sl = slice(ch * 512, (ch + 1) * 512)
                        sqc = ssc.tile([5, 512], f32, tag="sqc", name="sqc")
                        nc.vector.tensor_mul(sqc[:], src[:, sl], src[:, sl])
                        pssq = psmall.tile([1, 512], f32, tag="pss", name="pssq")
                        nc.tensor.matmul(pssq[:], ones5[:], sqc[:])
                        msqf = ssc.tile([1, 512], f32, tag="msqf", name="msqf")
                        nc.scalar.mul(msqf[:], pssq[:], -0.5)
                        mh = ssc.tile([1, 512], f16, tag="mh", name="mh")
                        nc.vector.tensor_copy(mh[:], msqf[:])
                        mhb = ssc.tile([1, 512], f32, tag="mhb", name="mhb")
                        nc.vector.tensor_copy(mhb[:], mh[:])
                        ml32 = ssc.tile([1, 512], f32, tag="ml32", name="ml32")
                        nc.vector.tensor_sub(ml32[:], msqf[:], mhb[:])
                        ml = ssc.tile([1, 512], f16, tag="ml", name="ml")
                        nc.vector.tensor_copy(ml[:], ml32[:])
                        nc.sync.dma_start(dst[hi_row:hi_row + 1, sl], mh[:])
                        nc.sync.dma_start(dst[lo_row:lo_row + 1, sl], ml[:])

                msq_rows(F5, N, Hs16, 17, 18)
                msq_rows(FB5, BLK, G16, 15, 16)

                # constant rows
                nc.sync.dma_start(Hs16[15:16, :], cv16_d[0:1, :])
                nc.sync.dma_start(Hs16[16:17, :], cv16_d[0:1, :])
                nc.sync.dma_start(Hs16[19:20, :], cv16_d[1:2, :])
                nc.sync.dma_start(Hs16[20:21, :], cv16_d[1:2, :])
                nc.sync.dma_start(G16[17:18, :], cv16_d[0:1, 0:BLK])
                nc.sync.dma_start(G16[18:19, :], cv16_d[0:1, 0:BLK])
                nc.sync.dma_start(G16[19:20, :], cv16_d[2:3, 0:BLK])
                nc.sync.dma_start(G16[20:21, :], cv16_d[2:3, 0:BLK])

                # unaries and q init
                nc.sync.dma_start(U0m[:], uin_d[0])
                nc.sync.dma_start(U1m[:], uin_d[1])
                nc.vector.tensor_copy(q0[:], U0m[:])
                nc.vector.tensor_copy(q1[:], U1m[:])

                # coefficients: A = cm@(sw[:,0]-sw[:,1]), B = cm@(bw[:,0]-bw[:,1]),
                # Cc = cm@(sw[:,1]+bw[:,1]);  q_c = (U_c - Cc_c) - A_c*sp0 - B_c*bl0n
                swt = ssb.tile([2, 2], f32)
                bwt = ssb.tile([2, 2], f32)
                cmT = ssb.tile([2, 2], f32)
                nc.sync.dma_start(swt[:], sw_d[:])
                nc.sync.dma_start(bwt[:], bw_d[:])
                nc.sync.dma_start(cmT[:], cm_d[:, :].rearrange("a b -> b a"))
                m3 = ssb.tile([2, 3], f32)
                nc.vector.tensor_sub(m3[:, 0:1], swt[:, 0:1], swt[:, 1:2])
                nc.vector.tensor_sub(m3[:, 1:2], bwt[:, 0:1], bwt[:, 1:2])
                nc.vector.tensor_add(m3[:, 2:3], swt[:, 1:2], bwt[:, 1:2])
                ps_c = psmall.tile([2, 3], f32, tag="pss", name="ps_c")
                nc.tensor.matmul(ps_c[:], cmT[:], m3[:])
                c23 = ssb.tile([2, 3], f32)
                nc.vector.tensor_copy(c23[:], ps_c[:])
                cflat_d = dramP.tile([1, 6], f32)
                nc.sync.dma_start(cflat_d[:], c23[:])
                cflat = ssb.tile([1, 6], f32)
                nc.sync.dma_start(cflat[:], cflat_d[:])
                ones_r = ssb.tile([1, 128], f32)
                nc.sync.dma_start(ones_r[:], cvec_d[0:1, 0:128])
                ps_b = psmall.tile([128, 6], f32, tag="pss", name="ps_b")
                nc.tensor.matmul(ps_b[:], ones_r[:], cflat[:])
                nc.vector.tensor_scalar_mul(negc[:], ps_b[:], -1.0)
                nc.vector.tensor_scalar(U0m[:], U0m[:], negc[:, 2:3], None, op0=ALU.add)
                nc.vector.tensor_scalar(U1m[:], U1m[:], negc[:, 5:6], None, op0=ALU.add)

                # K block build (fp16 -> DRAM) + bnorm
                bnflat = ssb.tile([1, BLK], f32)
                for g in range(NG):
                    psg = psbn.tile([1, 512], f32, tag="psbn", name="psg")
                    for pair in range(N // 256):
                        jt = 2 * pair
                        ps = psb.tile([128, 1024], f32, tag="psb", name="ps")
                        nc.tensor.matmul(
                            ps[:, 0:512],
                            Hs16[:, jt * 128:(jt + 1) * 128],
                            G16[:, g * 512:(g + 1) * 512],
                        )
                        nc.tensor.matmul(
                            ps[:, 512:1024],
                            Hs16[:, (jt + 1) * 128:(jt + 2) * 128],
                            G16[:, g * 512:(g + 1) * 512],
                        )
                        kt2 = k2p.tile([128, 1024], f16, tag="kt2", name="kt2")
                        nc.scalar.activation(kt2[:], ps[:], AF.Exp)
                        nc.tensor.matmul(
                            psg[:], ones16[:], kt2[:, 0:512],
                            start=(pair == 0), stop=False,
                        )
                        nc.tensor.matmul(
                            psg[:], ones16[:], kt2[:, 512:1024],
                            start=False, stop=(pair == N // 256 - 1),
                        )
                        nc.sync.dma_start(
                            K_dram[g, jt * 128:(jt + 1) * 128, :], kt2[:, 0:512]
                        )
                        nc.sync.dma_start(
                            K_dram[g, (jt + 1) * 128:(jt + 2) * 128, :], kt2[:, 512:1024]
                        )
                    nc.vector.tensor_copy(bnflat[:, g * 512:(g + 1) * 512], psg[:])

                # AllGather bnorm (row0 zeros, row1 bnorm) -> bnorm plane -> 1/bnorm
                agin0 = dram_ag.tile([2, BLK], f32, tag="agin", name="agin0")
                nc.sync.dma_start(agin0[0:1, :], cvec_d[1:2, 0:BLK])
                nc.sync.dma_start(agin0[1:2, :], bnflat[:])
                agout0 = dram_ag.tile(
                    [2 * M, BLK], f32, tag="agout", name="agout0", addr_space="Shared"
                )
                nc.gpsimd.collective_compute(
                    "AllGather", ALU.bypass, replica_groups=rg,
                    ins=[agin0.opt()], outs=[agout0.opt()],
                )
                bnp = ssb.tile([H, W], f32)
                for r in range(M):
                    nc.sync.dma_start(
                        bnp[r * 16:(r + 1) * 16, :],
                        agout0[2 * r + 1:2 * r + 2, :].rearrange(
                            "o (a b) -> (o a) b", a=16
                        ),
                    )
                nc.vector.reciprocal(inv_bn[:], bnp[:])

            # ---------------- iterations ----------------
            with (
                tc.tile_pool(name="isb", bufs=2) as isb,
                tc.tile_pool(name="kstr", bufs=8) as kstr,
                tc.tile_pool(name="psact", bufs=2, space="PSUM") as psact,
                tc.tile_pool(name="psdma", bufs=2, space="PSUM") as psdma,
            ):
                for t in range(ITERS):
                    # softmax pieces: s0 = 1/(1+e^(q1-q0)), nls = log(1+e^(q1-q0))
                    D = isb.tile([H, W], f32, tag="D", name="D")
                    nc.vector.tensor_sub(D[:], q1[:], q0[:])
                    E = isb.tile([H, W], f32, tag="E", name="E")
                    nc.scalar.activation(E[:], D[:], AF.Exp)
                    Uu = isb.tile([H, W], f32, tag="Uu", name="Uu")
                    nc.vector.tensor_scalar_add(Uu[:], E[:], 1.0)
                    S0 = isb.tile([H, W], f32, tag="S0", name="S0")
                    nc.vector.reciprocal(S0[:], Uu[:])
                    NLS = isb.tile([H, W], f32, tag="NLS", name="NLS")
                    nc.scalar.activation(NLS[:], Uu[:], AF.Ln)
                    # split nls hi/lo fp16 and write into Hs16 rows 19/20
                    NLH = isb.tile([H, W], f16, tag="NLH", name="NLH")
                    nc.vector.tensor_copy(NLH[:], NLS[:])
                    NLHB = isb.tile([H, W], f32, tag="NLHB", name="NLHB")
                    nc.vector.tensor_copy(NLHB[:], NLH[:])
                    NLL32 = isb.tile([H, W], f32, tag="NLL32", name="NLL32")
                    nc.vector.tensor_sub(NLL32[:], NLS[:], NLHB[:])
                    NLL = isb.tile([H, W], f16, tag="NLL", name="NLL")
                    nc.vector.tensor_copy(NLL[:], NLL32[:])
                    nls_d = dramP.tile([2, N], f16, tag="nls_d", name="nls_d", bufs=2)
                    nc.sync.dma_start(nls_d[0:1, :], NLH[:])
                    nc.sync.dma_start(nls_d[1:2, :], NLL[:])
                    nc.sync.dma_start(Hs16[19:20, :], nls_d[0:1, :])
                    nc.sync.dma_start(Hs16[20:21, :], nls_d[1:2, :])

                    # s0 transposed (fp16) = streamed-matvec weights
                    ps_t = psmall.tile([128, 128], f32, tag="pss", name="ps_t")
                    nc.tensor.transpose(ps_t[:], S0[:], ident[:])
                    s0T = isb.tile([128, 128], f16, tag="s0T", name="s0T")
                    nc.vector.tensor_copy(s0T[:], ps_t[:])

                    # spatial: sp0 = (Gs @ S0 @ Gs) * inv_sn   (Gs symmetric)
                    ps1 = psmall.tile([128, 128], f32, tag="pss", name="ps1")
                    nc.tensor.matmul(ps1[:], gs[:], S0[:])
                    T1 = isb.tile([H, W], f32, tag="T1", name="T1")
                    nc.vector.tensor_copy(T1[:], ps1[:])
                    ps2 = psmall.tile([128, 128], f32, tag="pss", name="ps2")
                    nc.tensor.transpose(ps2[:], T1[:], ident[:])
                    T1t = isb.tile([H, W], f32, tag="T1t", name="T1t")
                    nc.vector.tensor_copy(T1t[:], ps2[:])
                    ps3 = psmall.tile([128, 128], f32, tag="pss", name="ps3")
                    nc.tensor.matmul(ps3[:], gs[:], T1t[:])
                    T2t = isb.tile([H, W], f32, tag="T2t", name="T2t")
                    nc.vector.tensor_copy(T2t[:], ps3[:])
                    ps4 = psmall.tile([128, 128], f32, tag="pss", name="ps4")
                    nc.tensor.transpose(ps4[:], T2t[:], ident[:])
                    SP0 = isb.tile([H, W], f32, tag="SP0", name="SP0")
                    nc.vector.tensor_mul(SP0[:], ps4[:], isn[:])

                    # bilateral ACT path: exp(-0.5 d2 + log s0_j) accumulated over j
                    acc = isb.tile([128, 16 * KCH], f32, tag="acc", name="acc")
                    for it in range(16):
                        for kk in range(KCH):
                            pa = psact.tile([128, 1024], f32, tag="pa", name="pa")
                            j0 = N_DMA + kk * 1024
                            nc.tensor.matmul(
                                pa[:, 0:512],
                                G16[:, it * 128:(it + 1) * 128],
                                Hs16[:, j0:j0 + 512],
                            )
                            nc.tensor.matmul(
                                pa[:, 512:1024],
                                G16[:, it * 128:(it + 1) * 128],
                                Hs16[:, j0 + 512:j0 + 1024],
                            )
                            nc.scalar.activation(
                                pa[:], pa[:], AF.Exp,
                                accum_out=acc[:, it * KCH + kk:it * KCH + kk + 1],
                            )
                    blact = isb.tile([128, 16], f32, tag="blact", name="blact")
                    nc.vector.reduce_sum(
                        blact[:],
                        acc[:, :].rearrange("p (a b) -> p a b", b=KCH),
                        axis=AX.X,
                    )

                    # bilateral DMA path: stream fp16 K tiles, matvec with s0T cols
                    bldma = isb.tile([1, BLK], f32, tag="bldma", name="bldma")
                    njt = N_DMA // 128
                    for g in range(NG):
                        pd = psdma.tile([1, 512], f32, tag="pd", name="pd")
                        for jt in range(njt):
                            kt = kstr.tile([128, 512], f16, tag="kt", name="kt")
                            nc.sync.dma_start(
                                kt[:], K_dram[g, jt * 128:(jt + 1) * 128, :]
                            )
                            nc.tensor.matmul(
                                pd[:],
                                s0T[:, jt:jt + 1],
                                kt[:],
                                start=(jt == 0), stop=(jt == njt - 1),
                            )
                        nc.vector.tensor_copy(bldma[:, g * 512:(g + 1) * 512], pd[:])

                    # gather: row0 = act part (transposed to flat order), row1 = dma part
                    ps_bt = psmall.tile([16, 128], f32, tag="pss", name="ps_bt")
                    nc.tensor.transpose(ps_bt[:], blact[:], ident[:])
                    tbact = isb.tile([16, 128], f32, tag="tbact", name="tbact")
                    nc.vector.tensor_copy(tbact[:], ps_bt[:])
                    agin = dram_ag.tile([2, BLK], f32, tag="agin", name="agin")
                    nc.sync.dma_start(
                        agin[0:1, :].rearrange("o (a b) -> (o a) b", a=16), tbact[:]
                    )
                    nc.sync.dma_start(agin[1:2, :], bldma[:])
                    agout = dram_ag.tile(
                        [2 * M, BLK], f32, tag="agout", name="agout",
                        addr_space="Shared",
                    )
                    nc.gpsimd.collective_compute(
                        "AllGather", ALU.bypass, replica_groups=rg,
                        ins=[agin.opt()], outs=[agout.opt()],
                    )
                    APL = isb.tile([H, W], f32, tag="APL", name="APL")
                    DPL = isb.tile([H, W], f32, tag="DPL", name="DPL")
                    for r in range(M):
                        nc.sync.dma_start(
                            APL[r * 16:(r + 1) * 16, :],
                            agout[2 * r:2 * r + 1, :].rearrange(
                                "o (a b) -> (o a) b", a=16
                            ),
                        )
                        nc.sync.dma_start(
                            DPL[r * 16:(r + 1) * 16, :],
                            agout[2 * r + 1:2 * r + 2, :].rearrange(
                                "o (a b) -> (o a) b", a=16
                            ),
                        )
                    BL = isb.tile([H, W], f32, tag="BL", name="BL")
                    nc.vector.tensor_add(BL[:], APL[:], DPL[:])
                    BLN = isb.tile([H, W], f32, tag="BLN", name="BLN")
                    nc.vector.tensor_mul(BLN[:], BL[:], inv_bn[:])

                    # q update: q_c = (U_c - Cc_c) - A_c*sp0 - B_c*bl0n
                    t0 = isb.tile([H, W], f32, tag="t0", name="t0")
                    nc.vector.scalar_tensor_tensor(
                        t0[:], SP0[:], negc[:, 0:1], U0m[:], op0=ALU.mult, op1=ALU.add
                    )
                    nc.vector.scalar_tensor_tensor(
                        q0[:], BLN[:], negc[:, 1:2], t0[:], op0=ALU.mult, op1=ALU.add
                    )
                    t1 = isb.tile([H, W], f32, tag="t1", name="t1")
                    nc.vector.scalar_tensor_tensor(
                        t1[:], SP0[:], negc[:, 3:4], U1m[:], op0=ALU.mult, op1=ALU.add
                    )
                    nc.vector.scalar_tensor_tensor(
                        q1[:], BLN[:], negc[:, 4:5], t1[:], op0=ALU.mult, op1=ALU.add
                    )

            nc.sync.dma_start(qout_d[0], q0[:])
            nc.sync.dma_start(qout_d[1], q1[:])

    nc.compile()
    return nc


def _get_nc():
    if "nc" not in _CACHE:
        _CACHE["nc"] = _build()
    return _CACHE["nc"]


def kernel(**inputs):
    from concourse.bass_utils import run_bass_kernel_spmd

    unaries = np.ascontiguousarray(np.asarray(inputs["unaries"], dtype=np.float32))
    rgb = np.ascontiguousarray(np.asarray(inputs["rgb"], dtype=np.float32))
    sw = np.ascontiguousarray(np.asarray(inputs["spatial_ker_weights"], dtype=np.float32))
    bw = np.ascontiguousarray(np.asarray(inputs["bilateral_ker_weights"], dtype=np.float32))
    cm = np.ascontiguousarray(np.asarray(inputs["compatibility_matrix"], dtype=np.float32))

    gsm = _gauss1d(H, TG)
    rsum = gsm.sum(axis=1).astype(np.float32)
    inv_sn = (1.0 / np.outer(rsum, rsum)).astype(np.float32)
    ident = np.eye(128, dtype=np.float32)
    ys, xs = np.meshgrid(
        np.arange(H, dtype=np.float32), np.arange(W, dtype=np.float32), indexing="ij"
    )
    gridT = np.ascontiguousarray(
        np.stack([(xs * (3.0 / TA)).reshape(N), (ys * (3.0 / TA)).reshape(N)])
    ).astype(np.float32)
    rgbT = np.ascontiguousarray(rgb[0].reshape(N, 3).T).astype(np.float32)
    uin = np.ascontiguousarray(unaries[0].transpose(2, 0, 1)).astype(np.float32)

    cvec = np.stack([
        np.ones(N, np.float32), np.zeros(N, np.float32), -np.ones(N, np.float32)
    ])
    cvec16 = cvec.astype(np.float16)
    common = {
        "gs": gsm, "inv_sn": inv_sn, "ident": ident, "gridT": gridT, "rgbT": rgbT,
        "uin": uin, "sw": sw, "bw": bw, "cm": cm, "cvec": cvec, "cvec16": cvec16,
    }
    in_maps = []
    for c in range(M):
        blk = slice(c * BLK, (c + 1) * BLK)
        m = dict(common)
        m["gridB"] = np.ascontiguousarray(gridT[:, blk])
        m["rgbB"] = np.ascontiguousarray(rgbT[:, blk])
        in_maps.append(m)

    nc = _get_nc()
    import os

    trace = bool(int(os.environ.get("BASS_KERNEL_TRACE", "0")))
    res = run_bass_kernel_spmd(nc, in_maps, list(range(M)), trace=trace)
    _CACHE["exec_time_ns"] = res.exec_time_ns
    _CACHE["results"] = res
    q = np.asarray(res.results[0]["qout"])  # [2, H, W]
    return np.ascontiguousarray(q.transpose(1, 2, 0))[None]  # (1, H, W, 2)


# revision 9
# speedup vs baseline: 2.2731x; 1.3016x over previous
"""Trainium2 Bass kernel for the CRF mean-field layer (nn_CrfLayer).

Algorithm (C=2 classes, H=W=128, N=16384 pixels, 10 mean-field iterations):
  - softmax over 2 classes == sigmoid; sum-to-one lets us filter only class 0:
    sp1 complement via spatial norm, bl1 = bnorm - bl0.
  - bilateral kernel K[i,j] = exp(-0.5*d2) via one augmented dot product on
    the PE; operands are split-fp16 (hi+lo) so the moving operand streams at
    full rate: exponent = f_i.f_j - 0.5|f_i|^2 - 0.5|f_j|^2 + log s0_j, so
    exp(psum) = K[i,j]*s0[j] directly (log s0 folded into the matmul).
  - sharding: core c owns columns i in [c*2048, (c+1)*2048) of K.  Setup
    builds that block once in fp16 into DRAM (and bnorm).  Each iteration the
    head of the j-range streams fp16 K tiles through the PE (moving operand)
    while the tail is recomputed on the fly with ScalarE exp+accumulate.  One
    32KB AllGather per iteration shares the per-core bl0 shards; the cheap
    per-pixel work (softmax, separable spatial filter, q update) is
    replicated on all cores.

split-fp16 contraction rows (k = 21), exponent = sum_k G[k,i] * Hs[k,j]:
   k0-4 : G fh_i    | Hs fh_j        k15: G msqh_i | Hs 1
   k5-9 : G fh_i    | Hs fl_j        k16: G msql_i | Hs 1
   k10-14: G fl_i   | Hs fh_j        k17: G 1      | Hs msqh_j
                                     k18: G 1      | Hs msql_j
                                     k19: G -1     | Hs nlsh_j   (-log s0 hi)
                                     k20: G -1     | Hs nlsl_j   (-log s0 lo)
"""

import sys

sys.path.insert(0, "/opt/trn_rl_repo")

import numpy as np

H = 128
W = 128
C = 2
N = H * W
M = 8
BLK = N // M  # 2048
TA, TB, TG = 160.0, 3.0, 3.0
ITERS = 10

# j-range 3-way split: [0, N_PE) streamed fp16 through the PE;
# [N_PE, N_PE+N_DVE) streamed fp16 through VectorE (tensor_tensor_reduce);
# [N_PE+N_DVE, N) recomputed on the fly via PE matmul + ScalarE exp-accum.
N_PE = 5120
N_DVE = 6144
N_ACT = N - N_PE - N_DVE
ICH = 512  # i-chunk width for the streamed matvec (one PSUM bank)
NG = BLK // ICH  # 4 i-groups per core
KCH = N_ACT // 1024  # ScalarE exp chunks (1024 wide) per i-tile
NBB = (N_DVE + N_ACT) // 1024  # setup [i x j]-orientation build chunks
NDV2 = N_DVE // 2048  # DVE chunks per i-tile
KR = 21  # contraction rows

_CACHE = {}


def _gauss1d(n, theta):
    d = np.arange(n, dtype=np.float32)
    return np.exp(-0.5 * ((d[:, None] - d[None, :]) / theta) ** 2).astype(np.float32)


def _build():
    import concourse.bass as bass
    import concourse.bacc as bacc
    from concourse import mybir, tile

    f32 = mybir.dt.float32
    f16 = mybir.dt.float16
    AF = mybir.ActivationFunctionType
    ALU = mybir.AluOpType
    AX = mybir.AxisListType

    nc = bacc.Bacc("TRN2", target_bir_lowering=False, debug=False, num_devices=M)

    gs_d = nc.declare_dram_parameter("gs", [H, H], f32, isOutput=False)
    isn_d = nc.declare_dram_parameter("inv_sn", [H, W], f32, isOutput=False)
    ident_d = nc.declare_dram_parameter("ident", [128, 128], f32, isOutput=False)
    grid_d = nc.declare_dram_parameter("gridT", [2, N], f32, isOutput=False)
    rgbT_d = nc.declare_dram_parameter("rgbT", [3, N], f32, isOutput=False)
    gridB_d = nc.declare_dram_parameter("gridB", [2, BLK], f32, isOutput=False)
    rgbB_d = nc.declare_dram_parameter("rgbB", [3, BLK], f32, isOutput=False)
    uin_d = nc.declare_dram_parameter("uin", [2, H, W], f32, isOutput=False)
    sw_d = nc.declare_dram_parameter("sw", [2, 2], f32, isOutput=False)
    bw_d = nc.declare_dram_parameter("bw", [2, 2], f32, isOutput=False)
    cm_d = nc.declare_dram_parameter("cm", [2, 2], f32, isOutput=False)
    cvec_d = nc.declare_dram_parameter("cvec", [3, N], f32, isOutput=False)
    cv16_d = nc.declare_dram_parameter("cvec16", [3, N], f16, isOutput=False)
    qout_d = nc.declare_dram_parameter("qout", [2, H, W], f32, isOutput=True)

    rg = [list(range(M))]

    with tile.TileContext(nc) as tc:
        with (
            tc.tile_pool(name="pers", bufs=1) as pers,
            tc.tile_pool(name="dramP", bufs=1, space="DRAM") as dramP,
            tc.tile_pool(name="dram_ag", bufs=2, space="DRAM") as dram_ag,
            tc.tile_pool(name="psmall", bufs=2, space="PSUM") as psmall,
        ):
            gs = pers.tile([H, H], f32)
            isn = pers.tile([H, W], f32)
            ident = pers.tile([128, 128], f32)
            Hs16 = pers.tile([KR, N], f16)
            G16 = pers.tile([KR, BLK], f16)
            negc = pers.tile([128, 6], f32)
            U0m = pers.tile([H, W], f32)
            U1m = pers.tile([H, W], f32)
            q0 = pers.tile([H, W], f32)
            q1 = pers.tile([H, W], f32)
            inv_bn = pers.tile([H, W], f32)
            ones16 = pers.tile([128, 1], f16)
            ones16r = pers.tile([1, 128], f16)

            K_dram = dramP.tile([NG, N_PE, ICH], f16)
            K2_dram = dramP.tile([16, 128, N_DVE], f16)

            nc.sync.dma_start(gs[:], gs_d[:])
            nc.sync.dma_start(isn[:], isn_d[:])
            nc.sync.dma_start(ident[:], ident_d[:])
            nc.sync.dma_start(ones16[:], cv16_d[0:1, 0:128].rearrange("a b -> b a"))
            nc.sync.dma_start(ones16r[:], cv16_d[0:1, 0:128])

            # ---------------- setup ----------------
            with (
                tc.tile_pool(name="ssb", bufs=1) as ssb,
                tc.tile_pool(name="ssc", bufs=2) as ssc,
                tc.tile_pool(name="psb", bufs=2, space="PSUM") as psb,
                tc.tile_pool(name="psbn", bufs=2, space="PSUM") as psbn,
                tc.tile_pool(name="k2p", bufs=4) as k2p,
            ):
                # f32 features (mean-centered): rows = [3x/160, 3y/160, r, g, b]/3
                F5 = ssb.tile([5, N], f32)
                FB5 = ssb.tile([5, BLK], f32)
                fmean = ssb.tile([5, 1], f32)
                nc.sync.dma_start(F5[0:2, :], grid_d[:])
                nc.sync.dma_start(F5[2:5, :], rgbT_d[:])
                nc.vector.tensor_scalar_mul(F5[:], F5[:], 1.0 / 3.0)
                nc.vector.reduce_sum(fmean[:], F5[:], axis=AX.X)
                nc.vector.tensor_scalar_mul(fmean[:], fmean[:], 1.0 / N)
                nc.vector.tensor_scalar_sub(F5[:], F5[:], fmean[:])
                nc.sync.dma_start(FB5[0:2, :], gridB_d[:])
                nc.sync.dma_start(FB5[2:5, :], rgbB_d[:])
                nc.vector.tensor_scalar_mul(FB5[:], FB5[:], 1.0 / 3.0)
                nc.vector.tensor_scalar_sub(FB5[:], FB5[:], fmean[:])

                # split features into fp16 hi/lo and scatter into Hs16/G16 rows
                def split_rows(src, nsrc, dst, hi_rows, lo_rows):
                    nch = nsrc // 1024
                    for ch in range(nch):
                        sl = slice(ch * 1024, (ch + 1) * 1024)
                        hi = ssc.tile([5, 1024], f16, tag="sp_hi", name="hi")
                        nc.vector.tensor_copy(hi[:], src[:, sl])
                        hb = ssc.tile([5, 1024], f32, tag="sp_hb", name="hb")
                        nc.vector.tensor_copy(hb[:], hi[:])
                        lo32 = ssc.tile([5, 1024], f32, tag="sp_lo32", name="lo32")
                        nc.vector.tensor_sub(lo32[:], src[:, sl], hb[:])
                        lo = ssc.tile([5, 1024], f16, tag="sp_lo", name="lo")
                        nc.vector.tensor_copy(lo[:], lo32[:])
                        for r0 in hi_rows:
                            nc.sync.dma_start(dst[r0:r0 + 5, sl], hi[:])
                        for r0 in lo_rows:
                            nc.sync.dma_start(dst[r0:r0 + 5, sl], lo[:])

                split_rows(F5, N, Hs16, hi_rows=(0, 10), lo_rows=(5,))
                split_rows(FB5, BLK, G16, hi_rows=(0, 5), lo_rows=(10,))

                # msq rows: -0.5*|f|^2 split hi/lo
                ones5 = ssb.tile([5, 1], f32)
                nc.sync.dma_start(ones5[:], cvec_d[0:1, 0:5].rearrange("a b -> b a"))

                def msq_rows(src, nsrc, dst, hi_row, lo_row):
                    for ch in range(nsrc // 512):
                        sl = slice(ch * 512, (ch + 1) * 512)
                        sqc = ssc.tile([5, 512], f32, tag="sqc", name="sqc")
                        nc.vector.tensor_mul(sqc[:], src[:, sl], src[:, sl])
                        pssq = psmall.tile([1, 512], f32, tag="pss", name="pssq")
                        nc.tensor.matmul(pssq[:], ones5[:], sqc[:])
                        msqf = ssc.tile([1, 512], f32, tag="msqf", name="msqf")
                        nc.scalar.mul(msqf[:], pssq[:], -0.5)
                        mh = ssc.tile([1, 512], f16, tag="mh", name="mh")
                        nc.vector.tensor_copy(mh[:], msqf[:])
                        mhb = ssc.tile([1, 512], f32, tag="mhb", name="mhb")
                        nc.vector.tensor_copy(mhb[:], mh[:])
                        ml32 = ssc.tile([1, 512], f32, tag="ml32", name="ml32")
                        nc.vector.tensor_sub(ml32[:], msqf[:], mhb[:])
                        ml = ssc.tile([1, 512], f16, tag="ml", name="ml")
                        nc.vector.tensor_copy(ml[:], ml32[:])
                        nc.sync.dma_start(dst[hi_row:hi_row + 1, sl], mh[:])
                        nc.sync.dma_start(dst[lo_row:lo_row + 1, sl], ml[:])

                msq_rows(F5, N, Hs16, 17, 18)
                msq_rows(FB5, BLK, G16, 15, 16)

                # constant rows
                nc.sync.dma_start(Hs16[15:16, :], cv16_d[0:1, :])
                nc.sync.dma_start(Hs16[16:17, :], cv16_d[0:1, :])
                nc.sync.dma_start(Hs16[19:20, :], cv16_d[1:2, :])
                nc.sync.dma_start(Hs16[20:21, :], cv16_d[1:2, :])
                nc.sync.dma_start(G16[17:18, :], cv16_d[0:1, 0:BLK])
                nc.sync.dma_start(G16[18:19, :], cv16_d[0:1, 0:BLK])
                nc.sync.dma_start(G16[19:20, :], cv16_d[2:3, 0:BLK])
                nc.sync.dma_start(G16[20:21, :], cv16_d[2:3, 0:BLK])

                # unaries and q init
                nc.sync.dma_start(U0m[:], uin_d[0])
                nc.sync.dma_start(U1m[:], uin_d[1])
                nc.vector.tensor_copy(q0[:], U0m[:])
                nc.vector.tensor_copy(q1[:], U1m[:])

                # coefficients: A = cm@(sw[:,0]-sw[:,1]), B = cm@(bw[:,0]-bw[:,1]),
                # Cc = cm@(sw[:,1]+bw[:,1]);  q_c = (U_c - Cc_c) - A_c*sp0 - B_c*bl0n
                swt = ssb.tile([2, 2], f32)
                bwt = ssb.tile([2, 2], f32)
                cmT = ssb.tile([2, 2], f32)
                nc.sync.dma_start(swt[:], sw_d[:])
                nc.sync.dma_start(bwt[:], bw_d[:])
                nc.sync.dma_start(cmT[:], cm_d[:, :].rearrange("a b -> b a"))
                m3 = ssb.tile([2, 3], f32)
                nc.vector.tensor_sub(m3[:, 0:1], swt[:, 0:1], swt[:, 1:2])
                nc.vector.tensor_sub(m3[:, 1:2], bwt[:, 0:1], bwt[:, 1:2])
                nc.vector.tensor_add(m3[:, 2:3], swt[:, 1:2], bwt[:, 1:2])
                ps_c = psmall.tile([2, 3], f32, tag="pss", name="ps_c")
                nc.tensor.matmul(ps_c[:], cmT[:], m3[:])
                c23 = ssb.tile([2, 3], f32)
                nc.vector.tensor_copy(c23[:], ps_c[:])
                cflat_d = dramP.tile([1, 6], f32)
                nc.sync.dma_start(cflat_d[:], c23[:])
                cflat = ssb.tile([1, 6], f32)
                nc.sync.dma_start(cflat[:], cflat_d[:])
                ones_r = ssb.tile([1, 128], f32)
                nc.sync.dma_start(ones_r[:], cvec_d[0:1, 0:128])
                ps_b = psmall.tile([128, 6], f32, tag="pss", name="ps_b")
                nc.tensor.matmul(ps_b[:], ones_r[:], cflat[:])
                nc.vector.tensor_scalar_mul(negc[:], ps_b[:], -1.0)
                nc.vector.tensor_scalar(U0m[:], U0m[:], negc[:, 2:3], None, op0=ALU.add)
                nc.vector.tensor_scalar(U1m[:], U1m[:], negc[:, 5:6], None, op0=ALU.add)

                # K block build (fp16 -> DRAM) + bnorm
                # phase A: [j x i] tiles for j in [0, N_PE) -> K_dram + PE bnorm
                bnflat = ssb.tile([1, BLK], f32)
                for g in range(NG):
                    psg = psbn.tile([1, 512], f32, tag="psbn", name="psg")
                    for pair in range(N_PE // 256):
                        jt = 2 * pair
                        ps = psb.tile([128, 1024], f32, tag="psb", name="ps")
                        nc.tensor.matmul(
                            ps[:, 0:512],
                            Hs16[:, jt * 128:(jt + 1) * 128],
                            G16[:, g * 512:(g + 1) * 512],
                        )
                        nc.tensor.matmul(
                            ps[:, 512:1024],
                            Hs16[:, (jt + 1) * 128:(jt + 2) * 128],
                            G16[:, g * 512:(g + 1) * 512],
                        )
                        kt2 = k2p.tile([128, 1024], f16, tag="kt2", name="kt2")
                        nc.scalar.activation(kt2[:], ps[:], AF.Exp)
                        nc.tensor.matmul(
                            psg[:], ones16[:], kt2[:, 0:512],
                            start=(pair == 0), stop=False,
                        )
                        nc.tensor.matmul(
                            psg[:], ones16[:], kt2[:, 512:1024],
                            start=False, stop=(pair == N_PE // 256 - 1),
                        )
                        nc.sync.dma_start(
                            K_dram[g, jt * 128:(jt + 1) * 128, :], kt2[:, 0:512]
                        )
                        nc.sync.dma_start(
                            K_dram[g, (jt + 1) * 128:(jt + 2) * 128, :], kt2[:, 512:1024]
                        )
                    nc.vector.tensor_copy(bnflat[:, g * 512:(g + 1) * 512], psg[:])

                # phase B: [i x j] tiles for j in [N_PE, N) -> K2_dram (DVE range
                # only) + ScalarE-accumulated bnorm partials
                bn_acc = ssb.tile([128, 16 * NBB], f32)
                for it in range(16):
                    for b in range(NBB):
                        j0 = N_PE + b * 1024
                        ps2 = psb.tile([128, 1024], f32, tag="psb", name="ps2")
                        nc.tensor.matmul(
                            ps2[:, 0:512],
                            G16[:, it * 128:(it + 1) * 128],
                            Hs16[:, j0:j0 + 512],
                        )
                        nc.tensor.matmul(
                            ps2[:, 512:1024],
                            G16[:, it * 128:(it + 1) * 128],
                            Hs16[:, j0 + 512:j0 + 1024],
                        )
                        kb = k2p.tile([128, 1024], f16, tag="kt2", name="kb")
                        nc.scalar.activation(
                            kb[:], ps2[:], AF.Exp,
                            accum_out=bn_acc[:, it * NBB + b:it * NBB + b + 1],
                        )
                        if b * 1024 < N_DVE:
                            nc.sync.dma_start(
                                K2_dram[it, :, b * 1024:(b + 1) * 1024], kb[:]
                            )
                bnact = ssb.tile([128, 16], f32)
                nc.vector.reduce_sum(
                    bnact[:],
                    bn_acc[:, :].rearrange("p (a b) -> p a b", b=NBB),
                    axis=AX.X,
                )
                ps_bn = psmall.tile([16, 128], f32, tag="pss", name="ps_bn")
                nc.tensor.transpose(ps_bn[:], bnact[:], ident[:])
                tbn = ssb.tile([16, 128], f32)
                nc.vector.tensor_copy(tbn[:], ps_bn[:])

                # AllGather bnorm (row0 = [i x j] part, row1 = PE part)
                agin0 = dram_ag.tile([2, BLK], f32, tag="agin", name="agin0")
                nc.sync.dma_start(
                    agin0[0:1, :].rearrange("o (a b) -> (o a) b", a=16), tbn[:]
                )
                nc.sync.dma_start(agin0[1:2, :], bnflat[:])
                agout0 = dram_ag.tile(
                    [2 * M, BLK], f32, tag="agout", name="agout0", addr_space="Shared"
                )
                nc.gpsimd.collective_compute(
                    "AllGather", ALU.bypass, replica_groups=rg,
                    ins=[agin0.opt()], outs=[agout0.opt()],
                )
                bnp = ssb.tile([H, W], f32)
                bnp2 = ssb.tile([H, W], f32)
                for r in range(M):
                    nc.sync.dma_start(
                        bnp[r * 16:(r + 1) * 16, :],
                        agout0[2 * r:2 * r + 1, :].rearrange(
                            "o (a b) -> (o a) b", a=16
                        ),
                    )
                    nc.sync.dma_start(
                        bnp2[r * 16:(r + 1) * 16, :],
                        agout0[2 * r + 1:2 * r + 2, :].rearrange(
                            "o (a b) -> (o a) b", a=16
                        ),
                    )
                nc.vector.tensor_add(bnp[:], bnp[:], bnp2[:])
                nc.vector.reciprocal(inv_bn[:], bnp[:])

            # ---------------- iterations ----------------
            with (
                tc.tile_pool(name="isb", bufs=2) as isb,
                tc.tile_pool(name="kstr", bufs=8) as kstr,
                tc.tile_pool(name="kdve", bufs=6) as kdve,
                tc.tile_pool(name="dacp", bufs=2) as dacp,
                tc.tile_pool(name="psact", bufs=2, space="PSUM") as psact,
                tc.tile_pool(name="psdma", bufs=2, space="PSUM") as psdma,
            ):
                for t in range(ITERS):
                    # softmax pieces: s0 = 1/(1+e^(q1-q0)), nls = log(1+e^(q1-q0))
                    D = isb.tile([H, W], f32, tag="D", name="D")
                    nc.vector.tensor_sub(D[:], q1[:], q0[:])
                    E = isb.tile([H, W], f32, tag="E", name="E")
                    nc.scalar.activation(E[:], D[:], AF.Exp)
                    Uu = isb.tile([H, W], f32, tag="Uu", name="Uu")
                    nc.vector.tensor_scalar_add(Uu[:], E[:], 1.0)
                    S0 = isb.tile([H, W], f32, tag="S0", name="S0")
                    nc.vector.reciprocal(S0[:], Uu[:])
                    NLS = isb.tile([H, W], f32, tag="NLS", name="NLS")
                    nc.scalar.activation(NLS[:], Uu[:], AF.Ln)
                    # split nls hi/lo fp16 and write into Hs16 rows 19/20
                    NLH = isb.tile([H, W], f16, tag="NLH", name="NLH")
                    nc.vector.tensor_copy(NLH[:], NLS[:])
                    NLHB = isb.tile([H, W], f32, tag="NLHB", name="NLHB")
                    nc.vector.tensor_copy(NLHB[:], NLH[:])
                    NLL32 = isb.tile([H, W], f32, tag="NLL32", name="NLL32")
                    nc.vector.tensor_sub(NLL32[:], NLS[:], NLHB[:])
                    NLL = isb.tile([H, W], f16, tag="NLL", name="NLL")
                    nc.vector.tensor_copy(NLL[:], NLL32[:])
                    nls_d = dramP.tile([2, N], f16, tag="nls_d", name="nls_d", bufs=2)
                    nc.sync.dma_start(nls_d[0:1, :], NLH[:])
                    nc.sync.dma_start(nls_d[1:2, :], NLL[:])
                    nc.sync.dma_start(Hs16[19:20, :], nls_d[0:1, :])
                    nc.sync.dma_start(Hs16[20:21, :], nls_d[1:2, :])

                    # s0 transposed (fp16) = streamed-matvec weights
                    ps_t = psmall.tile([128, 128], f32, tag="pss", name="ps_t")
                    nc.tensor.transpose(ps_t[:], S0[:], ident[:])
                    s0T = isb.tile([128, 128], f16, tag="s0T", name="s0T")
                    nc.vector.tensor_copy(s0T[:], ps_t[:])

                    # s0 broadcast across partitions (fp16) for the DVE range
                    S016 = isb.tile([H, W], f16, tag="S016", name="S016")
                    nc.vector.tensor_copy(S016[:], S0[:])
                    s0f_d = dramP.tile([1, N], f16, tag="s0f_d", name="s0f_d", bufs=2)
                    nc.sync.dma_start(s0f_d[:], S016[:])
                    s0flat = isb.tile([1, N], f16, tag="s0flat", name="s0flat")
                    nc.sync.dma_start(s0flat[:], s0f_d[:])
                    s0bc = isb.tile([128, N_DVE], f16, tag="s0bc", name="s0bc")
                    for cb in range(N_DVE // 512):
                        psbc = psact.tile([128, 1024], f32, tag="pa", name="psbc")
                        nc.tensor.matmul(
                            psbc[:, 0:512], ones16r[:],
                            s0flat[0:1, N_PE + cb * 512:N_PE + (cb + 1) * 512],
                        )
                        nc.vector.tensor_copy(
                            s0bc[:, cb * 512:(cb + 1) * 512], psbc[:, 0:512]
                        )

                    # bilateral DVE path: streamed [i x j] fp16 K tiles * s0bc,
                    # chained multiply-reduce on VectorE
                    blacc_dve = isb.tile([128, 16], f32, tag="blacc_dve", name="blacc_dve")
                    for it in range(16):
                        accp = None
                        for cb in range(NDV2):
                            sl0 = cb * 2048
                            kdv = kdve.tile([128, 2048], f16, tag="kdv", name="kdv")
                            nc.sync.dma_start(
                                kdv[:], K2_dram[it, :, sl0:sl0 + 2048]
                            )
                            scr = isb.tile([128, 2048], f16, tag="scr", name="scr")
                            last = cb == NDV2 - 1
                            aout = dacp.tile([128, 1], f32, tag="dacc", name="dacc")
                            nc.vector.affine_mul_reduce(
                                scr[:], aout[:], kdv[:], s0bc[:, sl0:sl0 + 2048],
                                1.0, 0.0,
                            )
                            if accp is None:
                                accp = aout
                            else:
                                nxt = dacp.tile([128, 1], f32, tag="dacc2", name="dacc2")
                                if last:
                                    nc.vector.tensor_add(
                                        blacc_dve[:, it:it + 1], accp[:], aout[:]
                                    )
                                else:
                                    nc.vector.tensor_add(nxt[:], accp[:], aout[:])
                                    accp = nxt

                    # spatial: sp0 = (Gs @ S0 @ Gs) * inv_sn   (Gs symmetric)
                    ps1 = psmall.tile([128, 128], f32, tag="pss", name="ps1")
                    nc.tensor.matmul(ps1[:], gs[:], S0[:])
                    T1 = isb.tile([H, W], f32, tag="T1", name="T1")
                    nc.vector.tensor_copy(T1[:], ps1[:])
                    ps2 = psmall.tile([128, 128], f32, tag="pss", name="ps2")
                    nc.tensor.transpose(ps2[:], T1[:], ident[:])
                    T1t = isb.tile([H, W], f32, tag="T1t", name="T1t")
                    nc.vector.tensor_copy(T1t[:], ps2[:])
                    ps3 = psmall.tile([128, 128], f32, tag="pss", name="ps3")
                    nc.tensor.matmul(ps3[:], gs[:], T1t[:])
                    T2t = isb.tile([H, W], f32, tag="T2t", name="T2t")
                    nc.vector.tensor_copy(T2t[:], ps3[:])
                    ps4 = psmall.tile([128, 128], f32, tag="pss", name="ps4")
                    nc.tensor.transpose(ps4[:], T2t[:], ident[:])
                    SP0 = isb.tile([H, W], f32, tag="SP0", name="SP0")
                    nc.vector.tensor_mul(SP0[:], ps4[:], isn[:])

                    # bilateral ACT path: exp(-0.5 d2 + log s0_j) accumulated over j
                    acc = isb.tile([128, 16 * KCH], f32, tag="acc", name="acc")
                    for it in range(16):
                        for kk in range(KCH):
                            pa = psact.tile([128, 1024], f32, tag="pa", name="pa")
                            j0 = N_PE + N_DVE + kk * 1024
                            nc.tensor.matmul(
                                pa[:, 0:512],
                                G16[:, it * 128:(it + 1) * 128],
                                Hs16[:, j0:j0 + 512],
                            )
                            nc.tensor.matmul(
                                pa[:, 512:1024],
                                G16[:, it * 128:(it + 1) * 128],
                                Hs16[:, j0 + 512:j0 + 1024],
                            )
                            nc.scalar.activation(
                                pa[:], pa[:], AF.Exp,
                                accum_out=acc[:, it * KCH + kk:it * KCH + kk + 1],
                            )
                    blact = isb.tile([128, 16], f32, tag="blact", name="blact")
                    nc.vector.reduce_sum(
                        blact[:],
                        acc[:, :].rearrange("p (a b) -> p a b", b=KCH),
                        axis=AX.X,
                    )
                    nc.vector.tensor_add(blact[:], blact[:], blacc_dve[:])

                    # bilateral DMA path: stream fp16 K tiles, matvec with s0T cols
                    bldma = isb.tile([1, BLK], f32, tag="bldma", name="bldma")
                    njt = N_PE // 128
                    for g in range(NG):
                        pd = psdma.tile([1, 512], f32, tag="pd", name="pd")
                        for jt in range(njt):
                            kt = kstr.tile([128, 512], f16, tag="kt", name="kt")
                            nc.sync.dma_start(
                                kt[:], K_dram[g, jt * 128:(jt + 1) * 128, :]
                            )
                            nc.tensor.matmul(
                                pd[:],
                                s0T[:, jt:jt + 1],
                                kt[:],
                                start=(jt == 0), stop=(jt == njt - 1),
                            )
                        nc.vector.tensor_copy(bldma[:, g * 512:(g + 1) * 512], pd[:])

                    # gather: row0 = act part (transposed to flat order), row1 = dma part
                    ps_bt = psmall.tile([16, 128], f32, tag="pss", name="ps_bt")
                    nc.tensor.transpose(ps_bt[:], blact[:], ident[:])
                    tbact = isb.tile([16, 128], f32, tag="tbact", name="tbact")
                    nc.vector.tensor_copy(tbact[:], ps_bt[:])
                    agin = dram_ag.tile([2, BLK], f32, tag="agin", name="agin")
                    nc.sync.dma_start(
                        agin[0:1, :].rearrange("o (a b) -> (o a) b", a=16), tbact[:]
                    )
                    nc.sync.dma_start(agin[1:2, :], bldma[:])
                    agout = dram_ag.tile(
                        [2 * M, BLK], f32, tag="agout", name="agout",
                        addr_space="Shared",
                    )
                    nc.gpsimd.collective_compute(
                        "AllGather", ALU.bypass, replica_groups=rg,
                        ins=[agin.opt()], outs=[agout.opt()],
                    )
                    APL = isb.tile([H, W], f32, tag="APL", name="APL")
                    DPL = isb.tile([H, W], f32, tag="DPL", name="DPL")
                    for r in range(M):
                        nc.sync.dma_start(
                            APL[r * 16:(r + 1) * 16, :],
                            agout[2 * r:2 * r + 1, :].rearrange(
                                "o (a b) -> (o a) b", a=16
                            ),
                        )
                        nc.sync.dma_start(
                            DPL[r * 16:(r + 1) * 16, :],
                            agout[2 * r + 1:2 * r + 2, :].rearrange(
                                "o (a b) -> (o a) b", a=16
                            ),
                        )
                    BL = isb.tile([H, W], f32, tag="BL", name="BL")
                    nc.vector.tensor_add(BL[:], APL[:], DPL[:])
                    BLN = isb.tile([H, W], f32, tag="BLN", name="BLN")
                    nc.vector.tensor_mul(BLN[:], BL[:], inv_bn[:])

                    # q update: q_c = (U_c - Cc_c) - A_c*sp0 - B_c*bl0n
                    t0 = isb.tile([H, W], f32, tag="t0", name="t0")
                    nc.vector.scalar_tensor_tensor(
                        t0[:], SP0[:], negc[:, 0:1], U0m[:], op0=ALU.mult, op1=ALU.add
                    )
                    nc.vector.scalar_tensor_tensor(
                        q0[:], BLN[:], negc[:, 1:2], t0[:], op0=ALU.mult, op1=ALU.add
                    )
                    t1 = isb.tile([H, W], f32, tag="t1", name="t1")
                    nc.vector.scalar_tensor_tensor(
                        t1[:], SP0[:], negc[:, 3:4], U1m[:], op0=ALU.mult, op1=ALU.add
                    )
                    nc.vector.scalar_tensor_tensor(
                        q1[:], BLN[:], negc[:, 4:5], t1[:], op0=ALU.mult, op1=ALU.add
                    )

            nc.sync.dma_start(qout_d[0], q0[:])
            nc.sync.dma_start(qout_d[1], q1[:])

    nc.compile()
    return nc


def _get_nc():
    if "nc" not in _CACHE:
        _CACHE["nc"] = _build()
    return _CACHE["nc"]


def kernel(**inputs):
    from concourse.bass_utils import run_bass_kernel_spmd

    unaries = np.ascontiguousarray(np.asarray(inputs["unaries"], dtype=np.float32))
    rgb = np.ascontiguousarray(np.asarray(inputs["rgb"], dtype=np.float32))
    sw = np.ascontiguousarray(np.asarray(inputs["spatial_ker_weights"], dtype=np.float32))
    bw = np.ascontiguousarray(np.asarray(inputs["bilateral_ker_weights"], dtype=np.float32))
    cm = np.ascontiguousarray(np.asarray(inputs["compatibility_matrix"], dtype=np.float32))

    gsm = _gauss1d(H, TG)
    rsum = gsm.sum(axis=1).astype(np.float32)
    inv_sn = (1.0 / np.outer(rsum, rsum)).astype(np.float32)
    ident = np.eye(128, dtype=np.float32)
    ys, xs = np.meshgrid(
        np.arange(H, dtype=np.float32), np.arange(W, dtype=np.float32), indexing="ij"
    )
    gridT = np.ascontiguousarray(
        np.stack([(xs * (3.0 / TA)).reshape(N), (ys * (3.0 / TA)).reshape(N)])
    ).astype(np.float32)
    rgbT = np.ascontiguousarray(rgb[0].reshape(N, 3).T).astype(np.float32)
    uin = np.ascontiguousarray(unaries[0].transpose(2, 0, 1)).astype(np.float32)

    cvec = np.stack([
        np.ones(N, np.float32), np.zeros(N, np.float32), -np.ones(N, np.float32)
    ])
    cvec16 = cvec.astype(np.float16)
    common = {
        "gs": gsm, "inv_sn": inv_sn, "ident": ident, "gridT": gridT, "rgbT": rgbT,
        "uin": uin, "sw": sw, "bw": bw, "cm": cm, "cvec": cvec, "cvec16": cvec16,
    }
    in_maps = []
    for c in range(M):
        blk = slice(c * BLK, (c + 1) * BLK)
        m = dict(common)
        m["gridB"] = np.ascontiguousarray(gridT[:, blk])
        m["rgbB"] = np.ascontiguousarray(rgbT[:, blk])
        in_maps.append(m)

    nc = _get_nc()
    import os

    trace = bool(int(os.environ.get("BASS_KERNEL_TRACE", "0")))
    res = run_bass_kernel_spmd(nc, in_maps, list(range(M)), trace=trace)
    _CACHE["exec_time_ns"] = res.exec_time_ns
    _CACHE["results"] = res
    q = np.asarray(res.results[0]["qout"])  # [2, H, W]
    return np.ascontiguousarray(q.transpose(1, 2, 0))[None]  # (1, H, W, 2)
